# revision 43
# baseline (speedup 1.0000x reference)
"""ConvR (dense_cnn) Trainium2 kernel — 8-core vocab/tensor-parallel, fp16.

Strategy (per sharding hint): entity-embedding table + output scores are
column-sharded across 8 cores; the small conv/fc path is replicated (each core
computes the full 256-sample hidden, then scores its 12500-entity shard).

v4, evidence from v2/v3 traces:
  - DMA engine assignment: [128, N] and [100, N] partition shapes spread
    across all 16 SDMA engines; [101, N] strided loads land on ONE engine
    (v3: a single embT chunk DMA ran 28us).  So the scoring bias row is
    dropped entirely (reference bias is identically zero; host_prep falls
    back to all-host sigmoid+bias if it ever isn't) and embT ships as a
    clean [100, SH] fp16 tensor -> K=100 scoring contraction.
  - Each dma_start costs ~0.6-1.1us of serial HWDGE ring issuance, so the
    DMA count stays minimal (20 on sync, 2 tiny on gpsimd) in priority
    order: conv inputs (3 chunk pairs), w3, embT (3 chunks), outputs.
  - v3's SBUF->SBUF block-diag scatters (one engine each, 72B packets) and
    the strided fc rhs (2.3x slower matmuls: 480ns vs 203ns for N=256) are
    both reverted: padded hw-major p4 from HBM, contiguous fc rhs.
  - scoring evac is the phase pacer (PSUM fp32 reads: ACT 153G elem/s,
    DVE 123G elem/s, Pool can't touch PSUM): units alternate ACT/DVE 13:13
    (measured ~1.35us vs ~1.43us per unit); raw units get host sigmoid.
  - conv bn1 bias (B1) folds via contraction row 125 (r4 row = B1, p4 row
    = 1.0) as in v2; hT needs no ones row now.

Device matmuls (contraction on partitions):
  conv:   ps[100c, 180] = r4blk[128,100].T @ p4blk[128,180]          x52
  fc:     h[100j, 256b] += W_hw[100c,100j].T @ X_hw[100c,256b]       x36
  score:  s[128b, 500e] = hT[100,128b].T @ embT[100,500e]            x50
"""
import os
import sys

sys.path.insert(0, "/opt/trn_rl_repo")

import numpy as np
from contextlib import ExitStack

B = 256          # batch
E = 100          # embedding dim
NE = 100000      # entities
NCORES = 8
SH = NE // NCORES    # 12500 entities per core
G = 52               # conv groups (5 samples each; 52*5 = 260 >= 256)
GS = 5               # samples per conv group
NCH = 500            # scoring N-chunk (one PSUM bank)
NCI = SH // NCH      # 25 scoring chunks
CHK = 5              # scoring chunks per output DMA
QC = 2               # scoring chunks per PSUM tile / evac unit (2 banks).
                     # 4-chunk units amortize evac overhead better but leave
                     # only 2 PSUM tiles in flight and recycle-stall the PE
                     # (measured: scoring 18.2us -> 26.4us).
EPS = 1e-5

# scoring evac engine per unit (0 ACT-sigmoid, 1 DVE-raw-copy).  Measured
# rates are near-equal (ACT ~1.35us, DVE ~1.43us per unit), so alternate.
# unit u covers (m = u // 13, ci in [2*(u%13), 2*(u%13)+2))
def _unit_eng(u):
    return u % 2

_CACHE = {}


def _build():
    import concourse.bass as bass  # noqa: F401
    import concourse.tile as tile
    from concourse import bacc, mybir

    f32 = mybir.dt.float32
    f16 = mybir.dt.float16
    AF = mybir.ActivationFunctionType

    nc = bacc.Bacc("TRN2", target_bir_lowering=False, debug=False,
                   num_devices=NCORES)

    # r4 and p4 interleaved per group (280 cols each: 100 filter + 180
    # patch) so one DMA chunk delivers whole groups and half the ring
    # issuance (~0.7us serial per dma_start)
    rp_d = nc.dram_tensor("rp", [128, G * 280], f16, kind="ExternalInput").ap()
    w3_d = nc.dram_tensor("w3", [100, 3600], f16, kind="ExternalInput").ap()
    b2_d = nc.dram_tensor("b2c", [100, 1], f32, kind="ExternalInput").ap()
    embT_d = nc.dram_tensor("embT", [100, SH], f16, kind="ExternalInput").ap()
    scores_d = nc.dram_tensor("scores", [128, 2 * SH], f16,
                              kind="ExternalOutput").ap()

    with tile.TileContext(nc) as tc, ExitStack() as ctx:
        cpool = ctx.enter_context(tc.tile_pool(name="const", bufs=1))

        rp_t = cpool.tile([128, G * 280], f16, tag="rp")
        w3_t = cpool.tile([100, 3600], f16, tag="w3")
        b2_t = cpool.tile([100, 1], f32, tag="b2c")
        embT_t = cpool.tile([100, SH], f16, tag="embT")
        X_t = cpool.tile([100, 36 * B], f16, tag="X")
        hT_t = cpool.tile([100, B], f16, tag="hT")
        sb_t = cpool.tile([128, 2 * SH], f16, tag="sb")

        # tiny loads off the ring (SWDGE); b2 only needed at fc time
        nc.gpsimd.dma_start(b2_t[:], b2_d[:])

        # dummy sigmoid at t~0: ACT's first instruction pulls in the
        # 'sigmoid_and_others' table set (which also contains relu), so the
        # 1.5us ACT_TABLE_LOAD otherwise paid at scoring start — right on
        # the critical path — happens while the engines are still idle.
        scr_t = cpool.tile([100, 2], f16, tag="scr")
        nc.vector.memset(scr_t[:, 0:1], 0.0)
        nc.scalar.activation(scr_t[:, 1:2], scr_t[:, 0:1], AF.Sigmoid)

        # sync HWDGE ring in priority order: conv inputs in 4 chunks (first
        # small, for an early conv start); then fc weights; then embT in
        # scoring order; output DMAs are emitted from the scoring loop below.
        GC = (4, 14, 17, 17)
        g0 = 0
        for gn in GC:
            nc.sync.dma_start(rp_t[:, g0 * 280:(g0 + gn) * 280],
                              rp_d[:, g0 * 280:(g0 + gn) * 280])
            g0 += gn
        nc.sync.dma_start(w3_t[:], w3_d[:])
        for c0, c1 in ((0, 9 * NCH), (9 * NCH, 17 * NCH), (17 * NCH, SH)):
            nc.sync.dma_start(embT_t[:, c0:c1], embT_d[:, c0:c1])

        # conv: 52 block-diag matmuls, two groups sharing one PSUM bank
        # ([100, 360] <= 512 fp32) so ONE evacuation covers a group pair —
        # the conv phase is evac-overhead-bound (ACT ~0.55us / DVE ~0.38us
        # per instr), so halving the instr count paces conv ~2x faster.
        # p4 group cols hw-major (col = hw*5 + s): the evac into hw-major
        # X[c, hw*256 + b] writes 5-contiguous runs per group; the fc rhs
        # stays fully contiguous.
        X_v = X_t[:].rearrange("p (hw b) -> p hw b", b=B)
        conv_ctx = ExitStack()
        pconv = conv_ctx.enter_context(
            tc.tile_pool(name="pconv", bufs=4, space="PSUM"))
        for gp in range(G // 2):
            ga = 2 * gp
            pt = pconv.tile([100, 2 * GS * 36], f32, tag="pconv")
            for j in range(2):
                g = ga + j
                nc.tensor.matmul(
                    pt[:, j * 180:(j + 1) * 180],
                    rp_t[:, g * 280:g * 280 + 100],
                    rp_t[:, g * 280 + 100:(g + 1) * 280],
                    start=True, stop=True)
            if ga * GS + 2 * GS <= B:
                src = pt[:].rearrange("p (g2 hw s) -> p hw g2 s", g2=2, s=GS)
                dst = (X_v[:, :, ga * GS:ga * GS + 2 * GS]
                       .rearrange("p hw (g2 s) -> p hw g2 s", g2=2))
                if gp % 2 == 0:
                    nc.scalar.activation(dst, src, AF.Relu)
                else:
                    nc.vector.tensor_scalar_max(dst, src, 0.0)
            else:
                # ragged tail (groups 50: 5 samples, 51: 1 sample)
                for j in range(2):
                    g = ga + j
                    ns = min(GS, B - g * GS)
                    src = (pt[:, j * 180:(j + 1) * 180]
                           .rearrange("p (hw s) -> p hw s", s=GS)[:, :, 0:ns])
                    dst = X_v[:, :, g * GS:g * GS + ns]
                    if j == 0:
                        nc.scalar.activation(dst, src, AF.Relu)
                    else:
                        nc.vector.tensor_scalar_max(dst, src, 0.0)

        # fc: accumulate 36 matmuls into one PSUM tile -> hT
        pfc_pool = conv_ctx.enter_context(
            tc.tile_pool(name="pfc", bufs=1, space="PSUM"))
        pfc = pfc_pool.tile([100, B], f32, tag="pfc")
        for hw in range(36):
            nc.tensor.matmul(
                pfc[:],
                w3_t[:, hw * 100:(hw + 1) * 100],
                X_t[:, hw * B:(hw + 1) * B],
                start=(hw == 0), stop=(hw == 35))
        nc.scalar.activation(hT_t[:], pfc[:], AF.Relu, bias=b2_t[:, 0:1])
        conv_ctx.close()  # free conv/fc PSUM banks for the scoring pool

        # scoring, m-outer; ci pairs share one 2-bank PSUM tile; 4 bufs keep
        # the matmul pipeline deep enough that evac never recycle-stalls it.
        psc = ctx.enter_context(tc.tile_pool(name="psc", bufs=4, space="PSUM"))
        NW = SH // (CHK * NCH)  # output DMAs per half with a final remainder
        for m in range(2):
            nout = 0
            for cp in range(0, NCI, QC):
                nq = min(QC, NCI - cp)
                ncol = NCH * nq
                ps = psc.tile([128, QC * 512], f32, tag="psc")
                for j in range(nq):
                    nc.tensor.matmul(
                        ps[:, j * 512:j * 512 + NCH],
                        hT_t[:, m * 128:(m + 1) * 128],
                        embT_t[:, (cp + j) * NCH:(cp + j + 1) * NCH],
                        start=True, stop=True)
                dst3 = (sb_t[:, m * SH + cp * NCH:m * SH + cp * NCH + ncol]
                        .rearrange("p (j c) -> p j c", c=NCH))
                src3 = (ps[:].rearrange("p (j c) -> p j c", c=512)
                        [:, 0:nq, 0:NCH])
                u = m * 13 + cp // QC
                if _unit_eng(u) == 0:
                    nc.scalar.activation(dst3, src3, AF.Sigmoid)
                else:
                    nc.vector.tensor_copy(dst3, src3)
                done_ci = cp + nq - 1
                # output chunk boundaries (ci): the last two are small so
                # the post-evacuation drain tail is short
                BND = (0, 5, 10, 15, 20, 23, 25)
                while nout + 1 < len(BND) and BND[nout + 1] - 1 <= done_ci:
                    c0 = m * SH + BND[nout] * NCH
                    c1 = m * SH + BND[nout + 1] * NCH
                    nc.sync.dma_start(scores_d[:, c0:c1], sb_t[:, c0:c1])
                    nout += 1

    nc.compile()
    return nc


def host_prep(inputs):
    f = {k: np.asarray(v) for k, v in inputs.items()}
    e1 = f['e1'].astype(np.int64)
    rel = f['rel'].astype(np.int64)
    e1e = np.ascontiguousarray(f['emb_e'][e1]).astype(np.float32)    # (B, 100)
    rg = np.ascontiguousarray(f['emb_rel'][rel]).astype(np.float32)  # (B, 2500)

    a0 = float(f['bn0_g'][0] / np.sqrt(f['bn0_v'][0] + EPS))
    b0 = float(f['bn0_b'][0] - f['bn0_m'][0] * a0)
    A1 = (f['bn1_g'] / np.sqrt(f['bn1_v'] + EPS)).astype(np.float32)
    B1 = (f['bn1_b'] - f['bn1_m'] * A1).astype(np.float32)
    s_rel = (f['bn_rel_g'] / np.sqrt(f['bn_rel_v'] + EPS)).astype(np.float32)
    t_rel = (f['bn_rel_b'] - f['bn_rel_m'] * s_rel).astype(np.float32)
    s_rel2 = s_rel * np.repeat(A1, 25)
    t_rel2 = t_rel * np.repeat(A1, 25)
    A2 = (f['bn2_g'] / np.sqrt(f['bn2_v'] + EPS)).astype(np.float32)
    B2p = ((f['fc_b'] - f['bn2_m']) * A2 + f['bn2_b']).astype(np.float32)

    # device path assumes the scoring bias is zero (it is, in the
    # reference); if not, skip device sigmoid and apply bias+sigmoid on host
    _CACHE['bias_nonzero'] = bool(np.any(f['bias']))

    # block-diag conv operands (5 samples per group, K rows 25s..25s+24):
    #   r4[25*s + k, g*100 + c] = rn[5g+s, c*25+k]       (dense slab stack)
    #   p4[25*s + k, g*180 + hw*5 + s] = patch[5g+s, hw, k]   (block-diag)
    rn = rg * s_rel2[None, :] + t_rel2[None, :]          # (B, 2500)
    rn = np.concatenate([rn, np.zeros((G * GS - B, 2500), np.float32)], 0)
    r4 = np.zeros((128, G * 100), np.float16)
    r4[:125] = (rn.reshape(G, GS, 100, 25).transpose(1, 3, 0, 2)
                .reshape(125, G * 100))
    r4[125] = np.tile(B1, G)          # bias row: pairs with p4 ones row
    x0 = e1e * a0 + b0
    win = np.lib.stride_tricks.sliding_window_view(
        x0.reshape(B, 10, 10), (5, 5), axis=(1, 2))      # (B,6,6,5,5)
    patch = win.reshape(B, 36, 25).astype(np.float32)    # (B, hw, k)
    patch = np.concatenate(
        [patch, np.zeros((G * GS - B, 36, 25), np.float32)], 0)
    p4 = np.zeros((128, G, GS * 36), np.float16)
    pg = patch.reshape(G, GS, 36, 25)                    # (g, s, hw, k)
    # group columns hw-major: col = hw*GS + s
    for s in range(GS):
        p4v = p4.reshape(128, G, 36, GS)
        p4v[25 * s:25 * s + 25, :, :, s] = pg[:, s].transpose(2, 0, 1)
    p4[125] = 1.0                     # bias row: pairs with r4 B1 row
    p4 = p4.reshape(128, G * GS * 36)

    rp = np.empty((128, G, 280), np.float16)
    rp[:, :, :100] = r4.reshape(128, G, 100)
    rp[:, :, 100:] = p4.reshape(128, G, 180)
    rp = np.ascontiguousarray(rp.reshape(128, G * 280))

    w3 = np.ascontiguousarray(
        (f['fc_w'].astype(np.float32) * A2[None, :]).reshape(100, 3600)
    ).astype(np.float16)
    embT = f['emb_e'].T.astype(np.float16)               # (100, NE)

    col = lambda v: np.ascontiguousarray(v.reshape(100, 1)).astype(np.float32)
    common = dict(rp=rp, w3=w3, b2c=col(B2p))
    in_maps = []
    for m in range(NCORES):
        d = dict(common)
        d['embT'] = np.ascontiguousarray(embT[:, m * SH:(m + 1) * SH])
        in_maps.append(d)
    return in_maps


def _get_nc():
    if 'nc' not in _CACHE:
        _CACHE['nc'] = _build()
    return _CACHE['nc']


def kernel(**inputs):
    from concourse import bass_utils
    from concourse.bass_interp import get_hw_module

    nc = _get_nc()
    in_maps = host_prep(inputs)

    kwargs = {}
    trace_dir = os.environ.get("CONVR_TRACE_DIR")
    if trace_dir:
        kwargs.update(tmpdir=trace_dir, trace=True)

    old_m = nc.m
    nc.m = get_hw_module(nc.m)
    try:
        res = bass_utils.run_bass_kernel_spmd(
            nc, in_maps, core_ids=list(range(NCORES)), **kwargs)
    finally:
        nc.m = old_m
    _CACHE['last_result'] = res

    # raw (non-sigmoided) column mask per output half, from the evac rotation
    raw_half = np.zeros((2, SH), bool)
    for m in range(2):
        for cp in range(0, NCI, QC):
            u = m * 13 + cp // QC
            if _unit_eng(u) != 0:
                nq = min(QC, NCI - cp)
                raw_half[m, cp * NCH:(cp + nq) * NCH] = True

    bias = np.asarray(inputs['bias']).astype(np.float32)
    bias_nz = _CACHE.get('bias_nonzero', False)

    out = np.empty((B, NE), np.float32)
    for m in range(NCORES):
        s = np.asarray(res.results[m]['scores']).astype(np.float32)
        s = s.reshape(128, 2, SH).transpose(1, 0, 2)     # (2, 128, SH)
        bsh = bias[m * SH:(m + 1) * SH]
        for h in range(2):
            if bias_nz:
                # device output is raw scores without bias for raw columns;
                # sigmoided-without-bias columns are recovered via logit
                sh = s[h]
                rc = raw_half[h]
                sh[:, ~rc] = np.log(np.clip(sh[:, ~rc], 1e-7, 1 - 1e-7) /
                                    (1 - np.clip(sh[:, ~rc], 1e-7, 1 - 1e-7)))
                sh += bsh[None, :]
                s[h] = 1.0 / (1.0 + np.exp(-sh))
            else:
                rc = raw_half[h]
                s[h][:, rc] = 1.0 / (1.0 + np.exp(-s[h][:, rc]))
        out[:, m * SH:(m + 1) * SH] = s.reshape(B, SH)
    return out
